# revision 43
# baseline (speedup 1.0000x reference)
"""Two-branch attention (self + cross) Bass kernel for 8 trn2 NeuronCores.

Data-parallel over batch: B=8 batches, one per core.  Per core:
  qkv1 = x1 @ qkv_w       (q1, k1 head-transposed layout; v1 natural)
  k2,v2 from x2 @ qkv_w[:, 768:]
  branch1: softmax(q1 k1^T * sc) v1 @ proj_w + proj_b
  branch2: softmax(q1 k2^T * sc) v2 @ proj_w + proj_b

V2 design notes (HW-measured costs in brackets):
  - q/k are stored fp8e4 in a DoubleRow-interleaved layout: per head-pair
    tile [64, 2, N]; partition p in [0,32) is head A, [32,64) head B, dim1
    holds the two 32-wide halves of hd.  Score matmuls run in DoubleRow
    perf mode (0.5 cycles/col, 107ns per [64x512]) with contract 2x32=64.
    The hd-halves interleave comes free from a per-128-col-block column
    permutation of qkv_w's q/k columns at the W cast (softmax scores are
    invariant to a shared permutation of q/k dims).  qkv production, v,
    and everything downstream of exp stay bf16: quantizing only at the
    score input keeps rel err ~1.5e-2 (double fp8 quantization measured
    3.0e-2, over the 2e-2 gate).
  - Scores are NOT pre-scaled; softmax scale rides the Exp's free scale
    immediate.  Exp [128,1024] from PSUM measured 1.18us; exp does NOT
    contend with PE psum writes, so attention is ACT-paced when other
    engines keep up.
  - DVE is the scarce engine on HW: PSUM-sourced f32 copies run 1x mode
    ((N+151)/0.96 ns), and reciprocal costs ~6 cycles/elem along the FREE
    dim only (a [1,512] recip is 3.2us; [128,512] is the same).  So the 4
    softmax denominators of a head-pair are gathered to partitions
    {0,32,64,96} of one tile and reciprocated in ONE op, then extracted to
    [1,512] tiles (partition_broadcast only reads the tile's physical
    partition 0) for gpsimd broadcast + DVE multiply on o^T.  Denominator
    comes from a ones-column appended to v (AV psum row 64).
  - Do NOT put psum readouts on ACT (scalar.copy thrashes the exp table
    set) and do NOT put tensor_tensor on Pool (2x slower than DVE).
  - Inputs arrive via few big DMAs (x: 4 quarters; W: 6 row-chunks + 2
    small priority column loads so head-pair 0 starts early; proj: 2);
    W/proj DMAs issue from the ACT queue, x/outs from SP.  W casts (with
    the column permutation) run on Pool.
  - Fills (x2 loads/transposes, k2/v2, qk slices, proj0) are split into
    ~0.3-0.7us pieces and emitted one per AV chunk via fill slots, so the
    in-order PE queue never carries a multi-us burst between heads.
"""

import numpy as np

import concourse.bass as bass
import concourse.mybir as mybir
from concourse import bacc
from concourse.tile import TileContext
from concourse.bass_utils import run_bass_kernel_spmd

B, N, C = 8, 1024, 768
H, HD = 12, 64
NT = N // 128    # 8 token chunks
CK = C // 128    # 6 contraction chunks of C
RP = CK // 2     # 3 contraction chunk-pairs (DoubleRow)
SCALE = HD ** -0.5
F32 = mybir.dt.float32
BF16 = mybir.dt.bfloat16
FP8 = mybir.dt.float8e4
EXP = mybir.ActivationFunctionType.Exp
DR = mybir.MatmulPerfMode.DoubleRow


def build(with_bias: bool, loop: int = 0, stages: str = "full"):
    nc = bacc.Bacc("TRN2", target_bir_lowering=False, debug=False, num_devices=8)
    x1_e = nc.declare_dram_parameter("x1", [N, C], F32, isOutput=False)
    x2_e = nc.declare_dram_parameter("x2", [N, C], F32, isOutput=False)
    w_e = nc.declare_dram_parameter("qkv_w", [C, 3 * C], F32, isOutput=False)
    p_e = nc.declare_dram_parameter("proj_w", [C, C], F32, isOutput=False)
    pb_e = nc.declare_dram_parameter("proj_b", [C], F32, isOutput=False)
    o1_e = nc.declare_dram_parameter("out1", [N, C], F32, isOutput=True)
    o2_e = nc.declare_dram_parameter("out2", [N, C], F32, isOutput=True)

    with TileContext(nc) as tc:
        with (
            tc.tile_pool(name="persist", bufs=1) as pp,
            tc.tile_pool(name="tmp", bufs=2) as tp,
            tc.tile_pool(name="attn", bufs=4) as atp,
            tc.tile_pool(name="small", bufs=4) as smp,
            tc.tile_pool(name="psum", bufs=1, space="PSUM") as ps,
        ):
            import contextlib
            loop_ctx = tc.For_i(0, loop, 1) if loop else contextlib.nullcontext()
            with loop_ctx:
                # ---- constants ----
                ones_bf = pp.tile([1, 128], BF16, tag="ones_bf")
                nc.gpsimd.memset(ones_bf[:], 1.0)
                if with_bias:
                    pb_f = pp.tile([1, C], F32, tag="pb_f")
                    nc.sync.dma_start(pb_f[:], pb_e[None, :])
                    pb_b = pp.tile([1, C], BF16, tag="pb_b")
                    nc.vector.tensor_copy(pb_b[:], pb_f[:])

                from concourse.masks import make_identity
                ident = pp.tile([128, 128], F32, tag="ident")
                make_identity(nc, ident)

                # ---- persistent weight / activation tiles ----
                # Wqk[r]: W rows r*128.. x (q|k cols 0:1536) bf16; within
                # each 128-col block the cols are permuted
                # [Ah0,Bh0,Ah1,Bh1] (32 each) for the DR interleave.
                Wqk = [pp.tile([128, 2 * C], BF16, tag=f"Wqk{r}",
                               name=f"Wqk{r}") for r in range(CK)]
                Wv = [pp.tile([128, C], BF16, tag=f"Wv{r}", name=f"Wv{r}")
                      for r in range(CK)]
                Pb = [pp.tile([128, C], BF16, tag=f"Pb{r}", name=f"Pb{r}")
                      for r in range(CK)]
                # transposed x: bf16 per c-chunk (v lhsT), fp8 DR pairs (qk rhs)
                xTb = {nm: [pp.tile([128, N], BF16, tag=f"{nm}Tb{c}",
                                    name=f"{nm}Tb{c}") for c in range(CK)]
                       for nm in ("x1", "x2")}
                # q/k head-pair tiles: [64, 2, N] fp8 (DoubleRow layout)
                qD = [pp.tile([64, 2, N], FP8, tag=f"qD{i}", name=f"qD{i}")
                      for i in range(6)]
                kD = [pp.tile([64, 2, N], FP8, tag=f"kD{i}", name=f"kD{i}")
                      for i in range(6)]
                k2D = [pp.tile([64, 2, N], FP8, tag=f"k2D{i}", name=f"k2D{i}")
                       for i in range(6)]
                vx = {nm: [pp.tile([128, H, HD + 1], BF16, tag=f"v_{nm}_{t}",
                                   name=f"v_{nm}_{t}") for t in range(NT)]
                      for nm in ("x1", "x2")}
                oT = {br: [pp.tile([128, N], BF16, tag=f"oT{br}_{c}",
                                   name=f"oT{br}_{c}") for c in range(CK)]
                      for br in (0, 1)}

                # ---- input loads (few big DMAs) ----
                def load_x_quarter(x_e, q, eng=None):
                    # [256, C] rows -> [128, 2, C] (partition = token % 128)
                    xl = tp.tile([128, 2 * C], F32, tag="ld32", bufs=2,
                                 name=f"xl{q}")
                    (eng or nc.sync).dma_start(
                        xl[:].rearrange("p (t c) -> p t c", c=C),
                        x_e[q * 256:(q + 1) * 256, :]
                        .rearrange("(t p) c -> p t c", p=128),
                    )
                    return xl

                def load_w_prio():
                    # priority loads: W cols for q slice m=0 and k slice m=6
                    # (all rows), so attention head-pair 0 can start early
                    for m, nm in ((0, "wq0"), (6, "wk0")):
                        wp = tp.tile([128, 6, 128], F32, tag="wp", bufs=1,
                                     name=nm)
                        nc.scalar.dma_start(
                            wp[:],
                            w_e[:, m * 128:(m + 1) * 128]
                            .rearrange("(r p) c -> p r c", p=128),
                        )
                        for r in range(CK):
                            nc.gpsimd.tensor_copy(
                                Wqk[r][:, m * 128:(m + 1) * 128]
                                .rearrange("p (t h s) -> p t h s",
                                           t=2, h=2, s=32),
                                wp[:, r, :].rearrange(
                                    "p (h t s) -> p t h s",
                                    h=2, t=2, s=32),
                            )

                def load_w_chunk(r):
                    wt = tp.tile([128, 3 * C], F32, tag="w32", bufs=2,
                                 name=f"wl{r}")
                    nc.scalar.dma_start(
                        wt[:], w_e[r * 128:(r + 1) * 128, :])
                    return wt

                def cast_w_chunk(r, wt):
                    # v part
                    nc.gpsimd.tensor_copy(Wv[r][:], wt[:, 2 * C:3 * C])
                    # qk part with per-128-block [h t s]->[t h s] permute;
                    # m=0 and m=6 already written by load_w_prio
                    for m0, mn in ((1, 5), (7, 5)):
                        nc.gpsimd.tensor_copy(
                            Wqk[r][:, m0 * 128:(m0 + mn) * 128]
                            .rearrange("p (m t h s) -> p m t h s",
                                       m=mn, t=2, h=2, s=32),
                            wt[:, m0 * 128:(m0 + mn) * 128].rearrange(
                                "p (m h t s) -> p m t h s",
                                m=mn, h=2, t=2, s=32),
                        )

                def load_proj_half(rr):
                    wt = tp.tile([128, 3 * C], F32, tag="w32", bufs=2,
                                 name=f"pl{rr}")
                    nc.scalar.dma_start(
                        wt[:].rearrange("p (r c) -> p r c", c=C),
                        p_e[rr * 384:(rr + 1) * 384, :]
                        .rearrange("(r p) c -> p r c", p=128),
                    )
                    for i in range(3):
                        nc.gpsimd.tensor_copy(
                            Pb[rr * 3 + i][:], wt[:, i * C:(i + 1) * C])

                def transpose_x(nm, t, xl):
                    transpose_piece(nm, t, xl, 0)
                    transpose_piece(nm, t, xl, 3)

                def qk_half(dst, m, src, j2, nm):
                    # half of a head-pair q/k slice: one [128, 512] psum
                    pt = ps.tile([128, 512], F32, tag="ps_q", bufs=2,
                                 name=f"qk{nm}_{j2}")
                    for c in range(CK):
                        nc.tensor.matmul(
                            pt[:],
                            lhsT=Wqk[c][:, m * 128:(m + 1) * 128],
                            rhs=src[c][:, j2 * 512:(j2 + 1) * 512],
                            start=(c == 0),
                            stop=(c == CK - 1),
                        )
                    jsl = slice(j2 * 512, (j2 + 1) * 512)
                    nc.vector.tensor_copy(dst[:, 0, jsl], pt[0:64, :])
                    nc.vector.tensor_copy(dst[:, 1, jsl], pt[64:128, :])

                def qk_slice(dst, m, src8, nm):
                    for j2 in range(2):
                        qk_half(dst, m, src8, j2, nm)

                def v_half(nm, t, i):
                    vt = vx[nm][t]
                    if i == 0:
                        nc.gpsimd.memset(vt[:, :, HD], 1.0)
                    n0, nw = ((0, 512), (512, 256))[i]
                    pt = ps.tile([128, nw], F32, tag="ps_q", bufs=2,
                                 name=f"vp{nm}{t}_{i}")
                    for c in range(CK):
                        nc.tensor.matmul(
                            pt[:],
                            lhsT=xTb[nm][c][:, t * 128:(t + 1) * 128],
                            rhs=Wv[c][:, n0:n0 + nw],
                            start=(c == 0),
                            stop=(c == CK - 1),
                        )
                    h0, h1 = n0 // HD, (n0 + nw) // HD
                    nc.vector.tensor_copy(
                        vt[:, h0:h1, 0:HD],
                        pt[:].rearrange("p (h d) -> p h d", d=HD),
                    )

                def v_chunk(nm, t):
                    v_half(nm, t, 0)
                    v_half(nm, t, 1)

                def transpose_piece(nm, t, xl, c0):
                    # 3 of the 6 per-t transposes; psum readout on ACT for
                    # x1 (head phase, ACT idle), DVE for x2 (ACT is pacing)
                    for c in range(c0, c0 + 3):
                        ptr = ps.tile([128, 128], F32, tag="ps_q", bufs=2,
                                      name=f"tr{nm}_{t}_{c}")
                        nc.tensor.transpose(
                            ptr[:], xl[:, (t % 2) * C + c * 128:
                                       (t % 2) * C + (c + 1) * 128], ident[:]
                        )
                        tsl = slice(t * 128, (t + 1) * 128)
                        nc.vector.tensor_copy(xTb[nm][c][:, tsl], ptr[:])

                def attn_pair(br, hp, fills=None):
                    kT = kD if br == 0 else k2D
                    v = vx["x1"] if br == 0 else vx["x2"]
                    kt_tile = kT[hp]
                    qt_tile = qD[hp]
                    # 4 denominators (hh x j) batched into one reciprocal
                    den4 = atp.tile([128, 512], F32, tag="den4", bufs=1,
                                    name=f"den{br}_{hp}")
                    den4r = atp.tile([128, 512], F32, tag="den4r", bufs=1,
                                     name=f"denr{br}_{hp}")
                    ot_uns = {}
                    for hh in range(2):
                        h = 2 * hp + hh
                        p0 = 32 * hh
                        r0 = hh * HD
                        pos = [
                            ps.tile([HD + 1, 512], F32, tag="ps_o", bufs=2,
                                    name=f"po{br}_{h}_{j}")
                            for j in range(2)
                        ]

                        def score_exp(c, p0=p0):
                            pt = ps.tile([128, N], F32, tag="ps_s",
                                         bufs=2, name=f"pt{br}_{h}_{c}")
                            for j in range(2):
                                nc.tensor.matmul(
                                    pt[:, j * 512:(j + 1) * 512],
                                    lhsT=kt_tile[p0:p0 + 32, :,
                                                 c * 128:(c + 1) * 128],
                                    rhs=qt_tile[p0:p0 + 32, :,
                                                j * 512:(j + 1) * 512],
                                    start=True,
                                    stop=True,
                                    perf_mode=DR,
                                )
                            at = atp.tile([128, N], BF16, tag="at", bufs=5,
                                          name=f"at{br}_{h}_{c}")
                            nc.scalar.activation(at[:], pt[:], EXP,
                                                 scale=SCALE)
                            return at

                        ats = score_exp(0)
                        for c in range(NT):
                            nxt = score_exp(c + 1) if c + 1 < NT else None
                            for j in range(2):
                                nc.tensor.matmul(
                                    pos[j][:],
                                    lhsT=v[c][:, h, :],
                                    rhs=ats[:, j * 512:(j + 1) * 512],
                                    start=(c == 0),
                                    stop=(c == NT - 1),
                                )
                            if fills is not None and c < NT - 1:
                                f = next(fills, None)
                                if f is not None:
                                    f()
                            ats = nxt
                        for j in range(2):
                            i4 = 32 * (2 * hh + j)
                            nc.vector.tensor_copy(
                                den4[i4:i4 + 1, :], pos[j][HD:HD + 1, :])
                            ot_un = ot_uns.get(("t", j))
                            if ot_un is None:
                                ot_un = atp.tile([128, 512], F32,
                                                 tag="ot_un", bufs=2,
                                                 name=f"otu{br}_{hp}_{j}")
                                ot_uns[("t", j)] = ot_un
                            nc.vector.tensor_copy(
                                ot_un[r0:r0 + HD, :], pos[j][0:HD, :]
                            )
                            ot_uns[(hh, j)] = ot_un
                        if fills is not None:
                            f = next(fills, None)
                            if f is not None:
                                f()
                    # one reciprocal covers all 4 denominator rows
                    nc.vector.reciprocal(den4r[:], den4[:])
                    for hh in range(2):
                        r0 = hh * HD
                        for j in range(2):
                            i4 = 32 * (2 * hh + j)
                            jsl = slice(j * 512, (j + 1) * 512)
                            # partition_broadcast only reads the tile's
                            # physical partition 0 -> extract row first
                            recf = smp.tile([1, 512], BF16, tag="recf",
                                            bufs=2,
                                            name=f"rec{br}_{hp}_{hh}{j}")
                            nc.vector.tensor_copy(recf[:],
                                                  den4r[i4:i4 + 1, :])
                            pbs_sb = atp.tile([128, 512], BF16, tag="pbs",
                                              bufs=2,
                                              name=f"pbs{br}_{hp}_{hh}{j}")
                            nc.gpsimd.partition_broadcast(
                                pbs_sb[:], recf[0:1, :]
                            )
                            nc.vector.tensor_tensor(
                                oT[br][hp][r0:r0 + HD, jsl],
                                ot_uns[(hh, j)][r0:r0 + HD, :],
                                pbs_sb[r0:r0 + HD, :],
                                mybir.AluOpType.mult,
                            )

                def proj_chunk(br, t):
                    o_e = o1_e if br == 0 else o2_e
                    ot = tp.tile([128, C], F32, tag="out_sb",
                                 name=f"out{br}_{t}")
                    for i, (n0, nw) in enumerate(((0, 512), (512, 256))):
                        pt = ps.tile([128, nw], F32, tag="ps_q", bufs=2,
                                     name=f"pj{br}_{t}_{i}")
                        for c in range(CK):
                            nc.tensor.matmul(
                                pt[:],
                                lhsT=oT[br][c][:, t * 128:(t + 1) * 128],
                                rhs=Pb[c][:, n0:n0 + nw],
                                start=(c == 0),
                                stop=(c == CK - 1) and not with_bias,
                            )
                        if with_bias:
                            nc.tensor.matmul(
                                pt[:], lhsT=ones_bf[:, 0:128],
                                rhs=pb_b[:, n0:n0 + nw],
                                start=False, stop=True,
                            )
                        nc.vector.tensor_copy(ot[:, n0:n0 + nw], pt[:])
                    nc.sync.dma_start(o_e[t * 128:(t + 1) * 128, :], ot[:])

                # ================= schedule =================
                # head phase: x1 + priority W cols, transposes, first q/k
                load_w_prio()
                for q in range(4):
                    xl = load_x_quarter(x1_e, q)
                    transpose_x("x1", 2 * q, xl)
                    transpose_x("x1", 2 * q + 1, xl)
                wts = [load_w_chunk(r) for r in range(CK)]
                for r in range(CK):
                    cast_w_chunk(r, wts[r])

                qk_slice(qD[0], 0, xTb["x1"], "q0")
                qk_slice(kD[0], 6, xTb["x1"], "k0")
                for t in range(NT):
                    v_chunk("x1", t)

                if stages == "qkv":
                    load_proj_half(0)
                    load_proj_half(1)
                    for q in range(4):
                        xl = load_x_quarter(x2_e, q)
                        transpose_x("x2", 2 * q, xl)
                        transpose_x("x2", 2 * q + 1, xl)
                    for m in range(1, 6):
                        qk_slice(qD[m], m, xTb["x1"], f"q{m}")
                        qk_slice(kD[m], 6 + m, xTb["x1"], f"k{m}")
                    for m in range(6):
                        qk_slice(k2D[m], 6 + m, xTb["x2"], f"kk{m}")
                    for t in range(NT):
                        v_chunk("x2", t)
                    for i in range(6):
                        nc.gpsimd.dma_start(o1_e[i:i + 1, :N - 256],
                                            qD[i][0:1, 0, 0:N - 256])
                        nc.gpsimd.dma_start(o1_e[6 + i:7 + i, :N - 256],
                                            kD[i][0:1, 0, 0:N - 256])
                        nc.gpsimd.dma_start(o2_e[i:i + 1, :N - 256],
                                            k2D[i][0:1, 0, 0:N - 256])
                    for t in range(NT):
                        nc.gpsimd.dma_start(o1_e[32 + t:33 + t, :H * HD],
                                            vx["x1"][t][0:1, :, 0:HD])
                        nc.gpsimd.dma_start(o2_e[32 + t:33 + t, :H * HD],
                                            vx["x2"][t][0:1, :, 0:HD])

                if stages != "qkv":
                    # ---- attention with per-chunk fill slots (16/pair) ----
                    x2q = {}

                    def qkf(dst, m, src8, j2, nm):
                        return lambda: qk_half(dst, m, src8, j2, nm)

                    def x2load(q):
                        def f():
                            x2q[q] = load_x_quarter(x2_e, q)
                        return f

                    def trf(t, c0):
                        return lambda: transpose_piece("x2", t, x2q[t // 2],
                                                       c0)

                    def vf(t):
                        return lambda: v_chunk("x2", t)

                    def qk_pair_fills(hp1):
                        # q/k1 slices for head-pair hp1 (4 pieces)
                        return [
                            qkf(qD[hp1], hp1, xTb["x1"], 0, f"q{hp1}_0"),
                            qkf(qD[hp1], hp1, xTb["x1"], 1, f"q{hp1}_1"),
                            qkf(kD[hp1], 6 + hp1, xTb["x1"], 0, f"k{hp1}_0"),
                            qkf(kD[hp1], 6 + hp1, xTb["x1"], 1, f"k{hp1}_1"),
                        ]

                    def plf(rr):
                        return lambda: load_proj_half(rr)

                    fills0 = []
                    fills0 += qk_pair_fills(1) + [x2load(0), x2load(1),
                                                  plf(0)] + [None] * 9
                    fills0 += qk_pair_fills(2) + [x2load(2), x2load(3),
                                                  plf(1)] \
                        + [trf(0, 0), trf(0, 3), trf(1, 0), trf(1, 3),
                           trf(2, 0), trf(2, 3), trf(3, 0), trf(3, 3)] \
                        + [None] * 1
                    fills0 += qk_pair_fills(3) \
                        + [trf(4, 0), trf(4, 3), trf(5, 0), trf(5, 3),
                           trf(6, 0), trf(6, 3), trf(7, 0), trf(7, 3)] \
                        + [None] * 4
                    fills0 += qk_pair_fills(4) \
                        + [vf(0), vf(1), vf(2), vf(3)] + [None] * 8
                    fills0 += qk_pair_fills(5) \
                        + [vf(4), vf(5), vf(6), vf(7),
                           qkf(k2D[0], 6, xTb["x2"], 0, "kk0_0"),
                           qkf(k2D[0], 6, xTb["x2"], 1, "kk0_1")] + [None] * 6
                    fills0 += [
                        qkf(k2D[1], 7, xTb["x2"], 0, "kk1_0"),
                        qkf(k2D[1], 7, xTb["x2"], 1, "kk1_1"),
                        qkf(k2D[2], 8, xTb["x2"], 0, "kk2_0"),
                        qkf(k2D[2], 8, xTb["x2"], 1, "kk2_1"),
                        qkf(k2D[3], 9, xTb["x2"], 0, "kk3_0"),
                        qkf(k2D[3], 9, xTb["x2"], 1, "kk3_1"),
                    ] + [None] * 10

                    it0 = iter(fills0)
                    for hp in range(6):
                        attn_pair(0, hp, it0)

                    if stages == "attn":
                        for i in range(4, 6):
                            qk_slice(k2D[i], 6 + i, xTb["x2"], f"kk{i}")
                        for hp in range(6):
                            attn_pair(1, hp)
                        for c in range(CK):
                            nc.gpsimd.dma_start(o1_e[c:c + 1, :N - 256],
                                                oT[0][c][0:1, 0:N - 256])
                            nc.gpsimd.dma_start(o2_e[c:c + 1, :N - 256],
                                                oT[1][c][0:1, 0:N - 256])
                    else:
                        # ---- attention br1 + k2 tail + proj br0 fills ----
                        def pjf(br, t):
                            return lambda: proj_chunk(br, t)

                        fills1 = []
                        fills1 += [
                            qkf(k2D[4], 10, xTb["x2"], 0, "kk4_0"),
                            qkf(k2D[4], 10, xTb["x2"], 1, "kk4_1"),
                            pjf(0, 0), pjf(0, 1),
                        ] + [None] * 12
                        fills1 += [
                            qkf(k2D[5], 11, xTb["x2"], 0, "kk5_0"),
                            qkf(k2D[5], 11, xTb["x2"], 1, "kk5_1"),
                            pjf(0, 2), pjf(0, 3),
                        ] + [None] * 12
                        for t in range(4, NT):
                            fills1 += [pjf(0, t)] + [None] * 15

                        it1 = iter(fills1)
                        for hp in range(6):
                            attn_pair(1, hp, it1)

                        # ---- proj br1 tail ----
                        for t in range(NT):
                            proj_chunk(1, t)

    nc.compile()
    return nc


_CACHE = {}


def _get_nc(with_bias: bool):
    if with_bias not in _CACHE:
        _CACHE[with_bias] = build(with_bias)
    return _CACHE[with_bias]


def kernel(x1, x2, qkv_w, proj_w, proj_b):
    x1 = np.ascontiguousarray(np.asarray(x1, dtype=np.float32))
    x2 = np.ascontiguousarray(np.asarray(x2, dtype=np.float32))
    qkv_w = np.ascontiguousarray(np.asarray(qkv_w, dtype=np.float32))
    proj_w = np.ascontiguousarray(np.asarray(proj_w, dtype=np.float32))
    proj_b = np.ascontiguousarray(np.asarray(proj_b, dtype=np.float32))

    with_bias = bool(np.any(proj_b))
    nc = _get_nc(with_bias)
    in_maps = [
        {"x1": x1[i], "x2": x2[i], "qkv_w": qkv_w, "proj_w": proj_w,
         "proj_b": proj_b}
        for i in range(B)
    ]
    res = run_bass_kernel_spmd(nc, in_maps, core_ids=list(range(B)))
    o1 = np.stack([res.results[i]["out1"] for i in range(B)])
    o2 = np.stack([res.results[i]["out2"] for i in range(B)])
    return (o1, o2)


# revision 45
# speedup vs baseline: 1.1712x; 1.1712x over previous
"""Two-branch attention (self + cross) Bass kernel for 8 trn2 NeuronCores.

Data-parallel over batch: B=8 batches, one per core.  Per core:
  qkv1 = x1 @ qkv_w       (q1, k1 head-transposed layout; v1 natural)
  k2,v2 from x2 @ qkv_w[:, 768:]
  branch1: softmax(q1 k1^T * sc) v1 @ proj_w + proj_b
  branch2: softmax(q1 k2^T * sc) v2 @ proj_w + proj_b

V2 design notes (HW-measured costs in brackets):
  - q/k are stored fp8e4 in a DoubleRow-interleaved layout: per head-pair
    tile [64, 2, N]; partition p in [0,32) is head A, [32,64) head B, dim1
    holds the two 32-wide halves of hd.  Score matmuls run in DoubleRow
    perf mode (0.5 cycles/col, 107ns per [64x512]) with contract 2x32=64.
    The hd-halves interleave comes free from a per-128-col-block column
    permutation of qkv_w's q/k columns at the W cast (softmax scores are
    invariant to a shared permutation of q/k dims).  qkv production, v,
    and everything downstream of exp stay bf16: quantizing only at the
    score input keeps rel err ~1.5e-2 (double fp8 quantization measured
    3.0e-2, over the 2e-2 gate).
  - Scores are NOT pre-scaled; softmax scale rides the Exp's free scale
    immediate.  Exp [128,1024] from PSUM measured 1.18us; exp does NOT
    contend with PE psum writes, so attention is ACT-paced when other
    engines keep up.
  - DVE is the scarce engine on HW: PSUM-sourced f32 copies run 1x mode
    ((N+151)/0.96 ns), and reciprocal costs ~6 cycles/elem along the FREE
    dim only (a [1,512] recip is 3.2us; [128,512] is the same).  So the 4
    softmax denominators of a head-pair are gathered to partitions
    {0,32,64,96} of one tile and reciprocated in ONE op, then extracted to
    [1,512] tiles (partition_broadcast only reads the tile's physical
    partition 0) for gpsimd broadcast + DVE multiply on o^T.  Denominator
    comes from a ones-column appended to v (AV psum row 64).
  - Do NOT put psum readouts on ACT (scalar.copy thrashes the exp table
    set) and do NOT put tensor_tensor on Pool (2x slower than DVE).
  - Inputs arrive via few big DMAs (x: 4 quarters; W: 6 row-chunks + 2
    small priority column loads so head-pair 0 starts early; proj: 2);
    W/proj DMAs issue from the ACT queue, x/outs from SP.  W casts (with
    the column permutation) run on Pool.
  - Fills (x2 loads/transposes, k2/v2, qk slices, proj0) are split into
    ~0.3-0.7us pieces and emitted one per AV chunk via fill slots, so the
    in-order PE queue never carries a multi-us burst between heads.
"""

import numpy as np

import concourse.bass as bass
import concourse.mybir as mybir
from concourse import bacc
from concourse.tile import TileContext
from concourse.bass_utils import run_bass_kernel_spmd

B, N, C = 8, 1024, 768
H, HD = 12, 64
NT = N // 128    # 8 token chunks
CK = C // 128    # 6 contraction chunks of C
RP = CK // 2     # 3 contraction chunk-pairs (DoubleRow)
SCALE = HD ** -0.5
F32 = mybir.dt.float32
BF16 = mybir.dt.bfloat16
FP8 = mybir.dt.float8e4
EXP = mybir.ActivationFunctionType.Exp
DR = mybir.MatmulPerfMode.DoubleRow


def build(with_bias: bool, loop: int = 0, stages: str = "full"):
    nc = bacc.Bacc("TRN2", target_bir_lowering=False, debug=False, num_devices=8)
    x1_e = nc.declare_dram_parameter("x1", [N, C], F32, isOutput=False)
    x2_e = nc.declare_dram_parameter("x2", [N, C], F32, isOutput=False)
    w_e = nc.declare_dram_parameter("qkv_w", [C, 3 * C], F32, isOutput=False)
    p_e = nc.declare_dram_parameter("proj_w", [C, C], F32, isOutput=False)
    pb_e = nc.declare_dram_parameter("proj_b", [C], F32, isOutput=False)
    o1_e = nc.declare_dram_parameter("out1", [N, C], F32, isOutput=True)
    o2_e = nc.declare_dram_parameter("out2", [N, C], F32, isOutput=True)

    with TileContext(nc) as tc:
        with (
            tc.tile_pool(name="persist", bufs=1) as pp,
            tc.tile_pool(name="tmp", bufs=2) as tp,
            tc.tile_pool(name="attn", bufs=4) as atp,
            tc.tile_pool(name="small", bufs=4) as smp,
            tc.tile_pool(name="psum", bufs=1, space="PSUM") as ps,
        ):
            import contextlib
            loop_ctx = tc.For_i(0, loop, 1) if loop else contextlib.nullcontext()
            with loop_ctx:
                # ---- constants ----
                ones_bf = pp.tile([1, 128], BF16, tag="ones_bf")
                nc.gpsimd.memset(ones_bf[:], 1.0)
                if with_bias:
                    pb_f = pp.tile([1, C], F32, tag="pb_f")
                    nc.sync.dma_start(pb_f[:], pb_e[None, :])
                    pb_b = pp.tile([1, C], BF16, tag="pb_b")
                    nc.vector.tensor_copy(pb_b[:], pb_f[:])

                from concourse.masks import make_identity
                ident = pp.tile([128, 128], F32, tag="ident")
                make_identity(nc, ident)

                # ---- persistent weight / activation tiles ----
                # Wqk[r]: W rows r*128.. x (q|k cols 0:1536) bf16; within
                # each 128-col block the cols are permuted
                # [Ah0,Bh0,Ah1,Bh1] (32 each) for the DR interleave.
                Wqk = [pp.tile([128, 2 * C], BF16, tag=f"Wqk{r}",
                               name=f"Wqk{r}") for r in range(CK)]
                Wv = [pp.tile([128, C], BF16, tag=f"Wv{r}", name=f"Wv{r}")
                      for r in range(CK)]
                Pb = [pp.tile([128, C], BF16, tag=f"Pb{r}", name=f"Pb{r}")
                      for r in range(CK)]
                # transposed x: bf16 per c-chunk (v lhsT), fp8 DR pairs (qk rhs)
                xTb = {nm: [pp.tile([128, N], BF16, tag=f"{nm}Tb{c}",
                                    name=f"{nm}Tb{c}") for c in range(CK)]
                       for nm in ("x1", "x2")}
                # q/k head-pair tiles: [64, 2, N] fp8 (DoubleRow layout)
                qD = [pp.tile([64, 2, N], FP8, tag=f"qD{i}", name=f"qD{i}")
                      for i in range(6)]
                kD = [pp.tile([64, 2, N], FP8, tag=f"kD{i}", name=f"kD{i}")
                      for i in range(6)]
                k2D = [pp.tile([64, 2, N], FP8, tag=f"k2D{i}", name=f"k2D{i}")
                       for i in range(6)]
                vx = {nm: [pp.tile([128, H, HD + 1], BF16, tag=f"v_{nm}_{t}",
                                   name=f"v_{nm}_{t}") for t in range(NT)]
                      for nm in ("x1", "x2")}
                oT = {br: [pp.tile([128, N], BF16, tag=f"oT{br}_{c}",
                                   name=f"oT{br}_{c}") for c in range(CK)]
                      for br in (0, 1)}

                # ---- input loads (few big DMAs) ----
                def load_x_quarter(x_e, q, eng=None):
                    # [256, C] rows -> [128, 2, C] (partition = token % 128)
                    xl = tp.tile([128, 2 * C], F32, tag="ld32", bufs=2,
                                 name=f"xl{q}")
                    (eng or nc.sync).dma_start(
                        xl[:].rearrange("p (t c) -> p t c", c=C),
                        x_e[q * 256:(q + 1) * 256, :]
                        .rearrange("(t p) c -> p t c", p=128),
                    )
                    return xl

                def load_w_prio():
                    # priority loads: W cols for q slice m=0 and k slice m=6
                    # (all rows), so attention head-pair 0 can start early
                    for m, nm in ((0, "wq0"), (6, "wk0")):
                        wp = tp.tile([128, 6, 128], F32, tag="wp", bufs=1,
                                     name=nm)
                        nc.scalar.dma_start(
                            wp[:],
                            w_e[:, m * 128:(m + 1) * 128]
                            .rearrange("(r p) c -> p r c", p=128),
                        )
                        for r in range(CK):
                            nc.gpsimd.tensor_copy(
                                Wqk[r][:, m * 128:(m + 1) * 128]
                                .rearrange("p (t h s) -> p t h s",
                                           t=2, h=2, s=32),
                                wp[:, r, :].rearrange(
                                    "p (h t s) -> p t h s",
                                    h=2, t=2, s=32),
                            )

                def load_w_chunk(r):
                    wt = tp.tile([128, 3 * C], F32, tag="w32", bufs=2,
                                 name=f"wl{r}")
                    nc.scalar.dma_start(
                        wt[:], w_e[r * 128:(r + 1) * 128, :])
                    return wt

                def cast_w_chunk(r, wt):
                    # v part
                    nc.gpsimd.tensor_copy(Wv[r][:], wt[:, 2 * C:3 * C])
                    # qk part with per-128-block [h t s]->[t h s] permute;
                    # m=0 and m=6 already written by load_w_prio
                    for m0, mn in ((1, 5), (7, 5)):
                        nc.gpsimd.tensor_copy(
                            Wqk[r][:, m0 * 128:(m0 + mn) * 128]
                            .rearrange("p (m t h s) -> p m t h s",
                                       m=mn, t=2, h=2, s=32),
                            wt[:, m0 * 128:(m0 + mn) * 128].rearrange(
                                "p (m h t s) -> p m t h s",
                                m=mn, h=2, t=2, s=32),
                        )

                def load_proj_half(rr):
                    wt = tp.tile([128, 3 * C], F32, tag="w32", bufs=2,
                                 name=f"pl{rr}")
                    nc.scalar.dma_start(
                        wt[:].rearrange("p (r c) -> p r c", c=C),
                        p_e[rr * 384:(rr + 1) * 384, :]
                        .rearrange("(r p) c -> p r c", p=128),
                    )
                    for i in range(3):
                        nc.gpsimd.tensor_copy(
                            Pb[rr * 3 + i][:], wt[:, i * C:(i + 1) * C])

                def transpose_x(nm, t, xl):
                    transpose_piece(nm, t, xl, 0)
                    transpose_piece(nm, t, xl, 3)

                def qk_half(dst, m, src, j2, nm):
                    # half of a head-pair q/k slice: one [128, 512] psum
                    pt = ps.tile([128, 512], F32, tag="ps_q", bufs=2,
                                 name=f"qk{nm}_{j2}")
                    for c in range(CK):
                        nc.tensor.matmul(
                            pt[:],
                            lhsT=Wqk[c][:, m * 128:(m + 1) * 128],
                            rhs=src[c][:, j2 * 512:(j2 + 1) * 512],
                            start=(c == 0),
                            stop=(c == CK - 1),
                        )
                    jsl = slice(j2 * 512, (j2 + 1) * 512)
                    nc.vector.tensor_copy(dst[:, 0, jsl], pt[0:64, :])
                    nc.vector.tensor_copy(dst[:, 1, jsl], pt[64:128, :])

                def qk_slice(dst, m, src8, nm):
                    for j2 in range(2):
                        qk_half(dst, m, src8, j2, nm)

                def v_half(nm, t, i):
                    vt = vx[nm][t]
                    if i == 0:
                        nc.gpsimd.memset(vt[:, :, HD], 1.0)
                    n0, nw = ((0, 512), (512, 256))[i]
                    pt = ps.tile([128, nw], F32, tag="ps_q", bufs=2,
                                 name=f"vp{nm}{t}_{i}")
                    for c in range(CK):
                        nc.tensor.matmul(
                            pt[:],
                            lhsT=xTb[nm][c][:, t * 128:(t + 1) * 128],
                            rhs=Wv[c][:, n0:n0 + nw],
                            start=(c == 0),
                            stop=(c == CK - 1),
                        )
                    h0, h1 = n0 // HD, (n0 + nw) // HD
                    nc.vector.tensor_copy(
                        vt[:, h0:h1, 0:HD],
                        pt[:].rearrange("p (h d) -> p h d", d=HD),
                    )

                def v_chunk(nm, t):
                    v_half(nm, t, 0)
                    v_half(nm, t, 1)

                def transpose_piece(nm, t, xl, c0):
                    # 3 of the 6 per-t transposes; psum readout on ACT for
                    # x1 (head phase, ACT idle), DVE for x2 (ACT is pacing)
                    for c in range(c0, c0 + 3):
                        ptr = ps.tile([128, 128], F32, tag="ps_q", bufs=2,
                                      name=f"tr{nm}_{t}_{c}")
                        nc.tensor.transpose(
                            ptr[:], xl[:, (t % 2) * C + c * 128:
                                       (t % 2) * C + (c + 1) * 128], ident[:]
                        )
                        tsl = slice(t * 128, (t + 1) * 128)
                        nc.vector.tensor_copy(xTb[nm][c][:, tsl], ptr[:])

                def attn_pair(br, hp, fills=None):
                    kT = kD if br == 0 else k2D
                    v = vx["x1"] if br == 0 else vx["x2"]
                    kt_tile = kT[hp]
                    qt_tile = qD[hp]
                    # 4 denominators (hh x j) batched into one reciprocal
                    den4 = atp.tile([128, 512], F32, tag="den4", bufs=1,
                                    name=f"den{br}_{hp}")
                    den4r = atp.tile([128, 512], F32, tag="den4r", bufs=1,
                                     name=f"denr{br}_{hp}")
                    ot_uns = {}
                    for hh in range(2):
                        h = 2 * hp + hh
                        p0 = 32 * hh
                        r0 = hh * HD
                        pos = [
                            ps.tile([HD + 1, 512], F32, tag="ps_o", bufs=2,
                                    name=f"po{br}_{h}_{j}")
                            for j in range(2)
                        ]

                        def score_exp(c, p0=p0):
                            pt = ps.tile([128, N], F32, tag="ps_s",
                                         bufs=2, name=f"pt{br}_{h}_{c}")
                            for j in range(2):
                                nc.tensor.matmul(
                                    pt[:, j * 512:(j + 1) * 512],
                                    lhsT=kt_tile[p0:p0 + 32, :,
                                                 c * 128:(c + 1) * 128],
                                    rhs=qt_tile[p0:p0 + 32, :,
                                                j * 512:(j + 1) * 512],
                                    start=True,
                                    stop=True,
                                    perf_mode=DR,
                                )
                            at = atp.tile([128, N], BF16, tag="at", bufs=5,
                                          name=f"at{br}_{h}_{c}")
                            nc.scalar.activation(at[:], pt[:], EXP,
                                                 scale=SCALE)
                            return at

                        ats = score_exp(0)
                        for c in range(NT):
                            nxt = score_exp(c + 1) if c + 1 < NT else None
                            for j in range(2):
                                nc.tensor.matmul(
                                    pos[j][:],
                                    lhsT=v[c][:, h, :],
                                    rhs=ats[:, j * 512:(j + 1) * 512],
                                    start=(c == 0),
                                    stop=(c == NT - 1),
                                )
                            if fills is not None and c < NT - 1:
                                f = next(fills, None)
                                if f is not None:
                                    f()
                            ats = nxt
                        for j in range(2):
                            i4 = 32 * (2 * hh + j)
                            nc.vector.tensor_copy(
                                den4[i4:i4 + 1, :], pos[j][HD:HD + 1, :])
                            ot_un = ot_uns.get(("t", j))
                            if ot_un is None:
                                ot_un = atp.tile([128, 512], F32,
                                                 tag="ot_un", bufs=2,
                                                 name=f"otu{br}_{hp}_{j}")
                                ot_uns[("t", j)] = ot_un
                            nc.vector.tensor_copy(
                                ot_un[r0:r0 + HD, :], pos[j][0:HD, :]
                            )
                            ot_uns[(hh, j)] = ot_un
                        if fills is not None:
                            f = next(fills, None)
                            if f is not None:
                                f()
                    # one reciprocal covers all 4 denominator rows
                    nc.vector.reciprocal(den4r[:], den4[:])
                    for hh in range(2):
                        r0 = hh * HD
                        for j in range(2):
                            i4 = 32 * (2 * hh + j)
                            jsl = slice(j * 512, (j + 1) * 512)
                            # partition_broadcast only reads the tile's
                            # physical partition 0 -> extract row first
                            recf = smp.tile([1, 512], BF16, tag="recf",
                                            bufs=2,
                                            name=f"rec{br}_{hp}_{hh}{j}")
                            nc.vector.tensor_copy(recf[:],
                                                  den4r[i4:i4 + 1, :])
                            pbs_sb = atp.tile([128, 512], BF16, tag="pbs",
                                              bufs=2,
                                              name=f"pbs{br}_{hp}_{hh}{j}")
                            nc.gpsimd.partition_broadcast(
                                pbs_sb[:], recf[0:1, :]
                            )
                            nc.vector.tensor_tensor(
                                oT[br][hp][r0:r0 + HD, jsl],
                                ot_uns[(hh, j)][r0:r0 + HD, :],
                                pbs_sb[r0:r0 + HD, :],
                                mybir.AluOpType.mult,
                            )

                def proj_chunk(br, t):
                    o_e = o1_e if br == 0 else o2_e
                    ot = tp.tile([128, C], F32, tag="out_sb",
                                 name=f"out{br}_{t}")
                    for i, (n0, nw) in enumerate(((0, 512), (512, 256))):
                        pt = ps.tile([128, nw], F32, tag="ps_q", bufs=2,
                                     name=f"pj{br}_{t}_{i}")
                        for c in range(CK):
                            nc.tensor.matmul(
                                pt[:],
                                lhsT=oT[br][c][:, t * 128:(t + 1) * 128],
                                rhs=Pb[c][:, n0:n0 + nw],
                                start=(c == 0),
                                stop=(c == CK - 1) and not with_bias,
                            )
                        if with_bias:
                            nc.tensor.matmul(
                                pt[:], lhsT=ones_bf[:, 0:128],
                                rhs=pb_b[:, n0:n0 + nw],
                                start=False, stop=True,
                            )
                        nc.vector.tensor_copy(ot[:, n0:n0 + nw], pt[:])
                    nc.sync.dma_start(o_e[t * 128:(t + 1) * 128, :], ot[:])

                # ================= schedule =================
                # head phase: x1 + priority W cols, transposes, first q/k
                load_w_prio()
                for q in range(4):
                    xl = load_x_quarter(x1_e, q)
                    transpose_x("x1", 2 * q, xl)
                    transpose_x("x1", 2 * q + 1, xl)
                wts = [load_w_chunk(r) for r in range(CK)]
                for r in range(CK):
                    cast_w_chunk(r, wts[r])

                qk_slice(qD[0], 0, xTb["x1"], "q0")
                qk_slice(kD[0], 6, xTb["x1"], "k0")
                for t in range(NT):
                    v_chunk("x1", t)

                if stages == "qkv":
                    load_proj_half(0)
                    load_proj_half(1)
                    for q in range(4):
                        xl = load_x_quarter(x2_e, q)
                        transpose_x("x2", 2 * q, xl)
                        transpose_x("x2", 2 * q + 1, xl)
                    for m in range(1, 6):
                        qk_slice(qD[m], m, xTb["x1"], f"q{m}")
                        qk_slice(kD[m], 6 + m, xTb["x1"], f"k{m}")
                    for m in range(6):
                        qk_slice(k2D[m], 6 + m, xTb["x2"], f"kk{m}")
                    for t in range(NT):
                        v_chunk("x2", t)
                    for i in range(6):
                        nc.gpsimd.dma_start(o1_e[i:i + 1, :N - 256],
                                            qD[i][0:1, 0, 0:N - 256])
                        nc.gpsimd.dma_start(o1_e[6 + i:7 + i, :N - 256],
                                            kD[i][0:1, 0, 0:N - 256])
                        nc.gpsimd.dma_start(o2_e[i:i + 1, :N - 256],
                                            k2D[i][0:1, 0, 0:N - 256])
                    for t in range(NT):
                        nc.gpsimd.dma_start(o1_e[32 + t:33 + t, :H * HD],
                                            vx["x1"][t][0:1, :, 0:HD])
                        nc.gpsimd.dma_start(o2_e[32 + t:33 + t, :H * HD],
                                            vx["x2"][t][0:1, :, 0:HD])

                if stages != "qkv":
                    # ---- attention with per-chunk fill slots (16/pair) ----
                    x2q = {}

                    def qkf(dst, m, src8, j2, nm):
                        return lambda: qk_half(dst, m, src8, j2, nm)

                    def x2load(q):
                        def f():
                            x2q[q] = load_x_quarter(x2_e, q)
                        return f

                    def trf(t, c0):
                        return lambda: transpose_piece("x2", t, x2q[t // 2],
                                                       c0)

                    def vf(t):
                        return lambda: v_chunk("x2", t)

                    def qk_pair_fills(hp1):
                        # q/k1 slices for head-pair hp1 (4 pieces)
                        return [
                            qkf(qD[hp1], hp1, xTb["x1"], 0, f"q{hp1}_0"),
                            qkf(qD[hp1], hp1, xTb["x1"], 1, f"q{hp1}_1"),
                            qkf(kD[hp1], 6 + hp1, xTb["x1"], 0, f"k{hp1}_0"),
                            qkf(kD[hp1], 6 + hp1, xTb["x1"], 1, f"k{hp1}_1"),
                        ]

                    def plf(rr):
                        return lambda: load_proj_half(rr)

                    fills0 = []
                    fills0 += qk_pair_fills(1) + [x2load(0), x2load(1),
                                                  plf(0)] + [None] * 9
                    fills0 += qk_pair_fills(2) + [x2load(2), x2load(3),
                                                  plf(1)] \
                        + [trf(0, 0), trf(0, 3), trf(1, 0), trf(1, 3),
                           trf(2, 0), trf(2, 3), trf(3, 0), trf(3, 3)] \
                        + [None] * 1
                    fills0 += qk_pair_fills(3) \
                        + [trf(4, 0), trf(4, 3), trf(5, 0), trf(5, 3),
                           trf(6, 0), trf(6, 3), trf(7, 0), trf(7, 3)] \
                        + [None] * 4
                    fills0 += qk_pair_fills(4) \
                        + [vf(0), vf(1), vf(2), vf(3)] + [None] * 8
                    fills0 += qk_pair_fills(5) \
                        + [vf(4), vf(5), vf(6), vf(7),
                           qkf(k2D[0], 6, xTb["x2"], 0, "kk0_0"),
                           qkf(k2D[0], 6, xTb["x2"], 1, "kk0_1")] + [None] * 6
                    fills0 += [
                        qkf(k2D[1], 7, xTb["x2"], 0, "kk1_0"),
                        qkf(k2D[1], 7, xTb["x2"], 1, "kk1_1"),
                        qkf(k2D[2], 8, xTb["x2"], 0, "kk2_0"),
                        qkf(k2D[2], 8, xTb["x2"], 1, "kk2_1"),
                        qkf(k2D[3], 9, xTb["x2"], 0, "kk3_0"),
                        qkf(k2D[3], 9, xTb["x2"], 1, "kk3_1"),
                    ] + [None] * 10

                    it0 = iter(fills0)
                    for hp in range(6):
                        attn_pair(0, hp, it0)

                    if stages == "attn":
                        for i in range(4, 6):
                            qk_slice(k2D[i], 6 + i, xTb["x2"], f"kk{i}")
                        for hp in range(6):
                            attn_pair(1, hp)
                        for c in range(CK):
                            nc.gpsimd.dma_start(o1_e[c:c + 1, :N - 256],
                                                oT[0][c][0:1, 0:N - 256])
                            nc.gpsimd.dma_start(o2_e[c:c + 1, :N - 256],
                                                oT[1][c][0:1, 0:N - 256])
                    else:
                        # ---- attention br1 + k2 tail + proj br0 fills ----
                        def pjf(br, t):
                            return lambda: proj_chunk(br, t)

                        fills1 = []
                        fills1 += [
                            qkf(k2D[4], 10, xTb["x2"], 0, "kk4_0"),
                            qkf(k2D[4], 10, xTb["x2"], 1, "kk4_1"),
                            pjf(0, 0), pjf(0, 1),
                        ] + [None] * 12
                        fills1 += [
                            qkf(k2D[5], 11, xTb["x2"], 0, "kk5_0"),
                            qkf(k2D[5], 11, xTb["x2"], 1, "kk5_1"),
                            pjf(0, 2), pjf(0, 3),
                        ] + [None] * 12
                        for t in range(4, NT):
                            fills1 += [pjf(0, t)] + [None] * 15

                        it1 = iter(fills1)
                        for hp in range(6):
                            attn_pair(1, hp, it1)

                        # ---- proj br1 tail ----
                        for t in range(NT):
                            proj_chunk(1, t)

    nc.compile()
    return nc


_CACHE = {}


def _get_nc(with_bias: bool):
    if with_bias not in _CACHE:
        _CACHE[with_bias] = build(with_bias)
    return _CACHE[with_bias]


def kernel(x1, x2, qkv_w, proj_w, proj_b):
    x1 = np.ascontiguousarray(np.asarray(x1, dtype=np.float32))
    x2 = np.ascontiguousarray(np.asarray(x2, dtype=np.float32))
    qkv_w = np.ascontiguousarray(np.asarray(qkv_w, dtype=np.float32))
    proj_w = np.ascontiguousarray(np.asarray(proj_w, dtype=np.float32))
    proj_b = np.ascontiguousarray(np.asarray(proj_b, dtype=np.float32))

    with_bias = bool(np.any(proj_b))
    nc = _get_nc(with_bias)
    in_maps = [
        {"x1": x1[i], "x2": x2[i], "qkv_w": qkv_w, "proj_w": proj_w,
         "proj_b": proj_b}
        for i in range(B)
    ]
    res = run_bass_kernel_spmd(nc, in_maps, core_ids=list(range(B)))
    o1 = np.stack([res.results[i]["out1"] for i in range(B)])
    o2 = np.stack([res.results[i]["out2"] for i in range(B)])
    return (o1, o2)
